# revision 6
# baseline (speedup 1.0000x reference)
"""Trainium2 Bass kernel for a 2-layer GNN (gather / scatter-sum message passing).

Math restructure (exact, fp32):
  layer(x) = x@W_self + b_self + Adj@x@Wx + EA@We + indeg*b_msg
where W_msg = [Wx (64 rows); We (16 rows)], Adj[n,m] = #edges m->n,
EA[n] = sum_{e: dst=n} edge_attr[e], indeg[n] = #edges into n.
EA/indeg are layer-independent (computed once, in the L1 scatter pass).

Sharding: nodes partitioned by dst across 8 cores (rows [6250c, 6250(c+1)));
gather tables (X, then H via AllGather) replicated per core.

Per core, per layer the hard op is AGG = Adj_local @ T for a table T [N,64]:
  - edges with dst in the core's range, sorted by dst, paired into G=2 slots
    per same-dst run (odd leftovers padded with a zero table row),
  - slots tiled into [128]-slot tiles, each tile inside one 128-node
    dst window; per tile: 2 indirect-DMA row gathers (128 rows each),
    pre-reduce add, one-hot(dstmod) built on-device, PE matmul
    (lhsT=payload [128,81|64], rhs=one-hot [128,128]) accumulated in PSUM
    over the window's tiles -> AGG^T feature-major.
Node-space matmuls run feature-major with weight matrices as stationary.
"""

import functools
import os
import numpy as np

N = 50000
E = 800000
NODE_IN = 64
EDGE_IN = 16
HID = 64
OUT = 32
BN_EPS = 1e-5

N_CORES = 8
NLOC = N // N_CORES            # 6250 nodes per core
WIN = 128                      # dst window (PSUM partition limit)
NWIN = (NLOC + WIN - 1) // WIN  # 49 windows per core
NLOC_PAD = NWIN * WIN          # 6272
G = 2                          # edges pre-reduced per slot
ZROW = N                       # zero row index in gather tables
TAB_ROWS = N + 64              # gather table rows (zero pad tail)
EAW = EDGE_IN + 1              # 16 edge feats + count column


# ----------------------------------------------------------------------------
# host-side prep: pure index manipulation / data layout
# ----------------------------------------------------------------------------

def _prep(edge_index, edge_attr):
    src = np.asarray(edge_index[0], dtype=np.int64)
    dst = np.asarray(edge_index[1], dtype=np.int64)

    order = np.argsort(dst, kind="stable")
    dst_s = dst[order]
    src_s = src[order]
    ea_s = np.asarray(edge_attr, dtype=np.float32)[order]

    core_bounds = np.searchsorted(dst_s, np.arange(N_CORES + 1) * NLOC)

    # per-edge position within its same-dst run
    cnt = np.bincount(dst_s, minlength=N)
    run_start = np.concatenate([[0], np.cumsum(cnt)[:-1]])
    pos = np.arange(E, dtype=np.int64) - run_start[dst_s]
    slot_in_node = pos // G
    member = pos % G

    slots_per_node = (cnt + G - 1) // G                     # [N]
    win_of_node = (np.arange(N) % NLOC) // WIN              # window within core
    node_core = np.arange(N) // NLOC
    # slots per (core, window)
    spw = np.zeros((N_CORES, NWIN), dtype=np.int64)
    np.add.at(spw, (node_core, win_of_node), slots_per_node)
    T_w = int(np.max((spw + 127) // 128))                   # tiles per window

    TOT = NWIN * T_w                                        # tiles per core
    # slot offset of each node within its window
    so = slots_per_node.copy()
    # cumsum within each (core, window) segment: windows are contiguous node
    # ranges, so a global cumsum reset at window starts works
    cum = np.cumsum(so)
    allnodes = np.arange(N)
    node_win_start = allnodes - ((allnodes % NLOC) % WIN)   # first node of window
    slot_off = cum - so - (cum[node_win_start] - so[node_win_start])

    idx_cols = np.full((N_CORES, 128, TOT * G), ZROW, dtype=np.int32)
    dst_cols = np.full((N_CORES, 128, TOT), -1.0, dtype=np.float32)
    ea_cols = np.zeros((N_CORES, 128, TOT * G * EAW), dtype=np.float32)

    c_of_e = dst_s // NLOC
    w_of_e = (dst_s % NLOC) // WIN
    slot_id = slot_off[dst_s] + slot_in_node                # within window
    k_loc = slot_id // 128
    p = slot_id % 128
    k = w_of_e * T_w + k_loc                                # tile within core

    idx_cols[c_of_e, p, k * G + member] = src_s.astype(np.int32)
    dst_cols[c_of_e, p, k] = ((dst_s % NLOC) % WIN).astype(np.float32)
    base = (k * G + member) * EAW
    for f in range(EDGE_IN):
        ea_cols[c_of_e, p, base + f] = ea_s[:, f]
    ea_cols[c_of_e, p, base + EDGE_IN] = 1.0
    return idx_cols, dst_cols, ea_cols, T_w


# ----------------------------------------------------------------------------
# device program
# ----------------------------------------------------------------------------

@functools.lru_cache(maxsize=2)
def _build(T_w):
    import concourse.bass as bass
    import concourse.mybir as mybir
    import concourse.tile as tile
    from concourse import bacc
    from concourse.masks import make_identity

    f32 = mybir.dt.float32
    i32 = mybir.dt.int32
    TOT = NWIN * T_w

    nc = bacc.Bacc("TRN2", target_bir_lowering=False, debug=False,
                   num_devices=N_CORES)

    P = nc.declare_dram_parameter
    xtab = P("xtab", [TAB_ROWS, NODE_IN], f32, isOutput=False)
    xt_loc = P("xt_loc", [NODE_IN, NLOC_PAD], f32, isOutput=False)
    idxs_d = P("idxs", [128, TOT * G], i32, isOutput=False)
    dstm_d = P("dstm", [128, TOT], f32, isOutput=False)
    ea_d = P("ea", [128, TOT * G * EAW], f32, isOutput=False)
    w1x_d = P("w1x", [NODE_IN, HID], f32, isOutput=False)
    w1sb_d = P("w1sb", [EAW + 1, HID], f32, isOutput=False)   # [W1e;b1m;b1s]
    w1s_d = P("w1s", [NODE_IN, HID], f32, isOutput=False)
    w2x_d = P("w2x", [HID, OUT], f32, isOutput=False)
    w2sb_d = P("w2sb", [EAW + 1, OUT], f32, isOutput=False)   # [W2e;b2m;b2s]
    w2s_d = P("w2s", [HID, OUT], f32, isOutput=False)
    bn_a_d = P("bn_a", [HID, 1], f32, isOutput=False)          # gamma*rsqrt(var+eps)
    bn_b_d = P("bn_b", [HID, 1], f32, isOutput=False)          # beta-mean*a
    out_d = P("out", [NLOC, OUT], f32, isOutput=True)

    with tile.TileContext(nc) as tc:
        with (
            tc.tile_pool(name="const", bufs=1) as cpool,
            tc.tile_pool(name="sb", bufs=4) as pool,
            tc.tile_pool(name="eapool", bufs=3) as eapool,
            tc.tile_pool(name="ps", bufs=3, space="PSUM") as psum,
            tc.tile_pool(name="psn", bufs=2, space="PSUM") as psumn,
            tc.tile_pool(name="pst", bufs=2, space="PSUM") as psumt,
            tc.tile_pool(name="dram", bufs=1, space="DRAM") as dram,
        ):
            # ---- constants ----
            iota_i = cpool.tile([128, 128], i32)
            nc.gpsimd.iota(iota_i[:], pattern=[[1, 128]], base=0,
                           channel_multiplier=0)
            iota_f = cpool.tile([128, 128], f32)
            nc.vector.tensor_copy(iota_f[:], iota_i[:])
            ident = cpool.tile([128, 128], f32)
            make_identity(nc, ident[:])

            idx_sb = cpool.tile([128, TOT * G], i32)
            nc.sync.dma_start(out=idx_sb[:], in_=idxs_d[:])
            dst_sb = cpool.tile([128, TOT], f32)
            nc.sync.dma_start(out=dst_sb[:], in_=dstm_d[:])
            xt_sb = cpool.tile([NODE_IN, NLOC_PAD], f32)
            nc.sync.dma_start(out=xt_sb[:], in_=xt_loc[:])
            w1x_sb = cpool.tile([NODE_IN, HID], f32)
            nc.sync.dma_start(out=w1x_sb[:], in_=w1x_d[:])
            w1sb_sb = cpool.tile([NODE_IN + EAW + 1, HID], f32)
            nc.sync.dma_start(out=w1sb_sb[NODE_IN:NODE_IN + EAW + 1, :],
                              in_=w1sb_d[:])
            w1s_sb = cpool.tile([NODE_IN, HID], f32)
            nc.sync.dma_start(out=w1s_sb[:], in_=w1s_d[:])
            w2x_sb = cpool.tile([HID, OUT], f32)
            nc.sync.dma_start(out=w2x_sb[:], in_=w2x_d[:])
            w2sb_sb = cpool.tile([NODE_IN + EAW + 1, OUT], f32)
            nc.sync.dma_start(out=w2sb_sb[NODE_IN:NODE_IN + EAW + 1, :],
                              in_=w2sb_d[:])
            w2s_sb = cpool.tile([HID, OUT], f32)
            nc.sync.dma_start(out=w2s_sb[:], in_=w2s_d[:])
            bn_a_sb = cpool.tile([HID, 1], f32)
            nc.sync.dma_start(out=bn_a_sb[:], in_=bn_a_d[:])
            bn_b_sb = cpool.tile([HID, 1], f32)
            nc.sync.dma_start(out=bn_b_sb[:], in_=bn_b_d[:])

            # AGG^T [agg_x(64) | EA(16) | cnt(1) | ones(1)] x NLOC_PAD
            aggt = cpool.tile([NODE_IN + EAW + 1, NLOC_PAD], f32)
            nc.vector.memset(aggt[NODE_IN:NODE_IN + EAW + 1, :], 1.0)
            agg2t = cpool.tile([HID, NLOC_PAD], f32)
            ht_sb = cpool.tile([HID, NLOC_PAD], f32)

            h_loc = nc.dram_tensor("h_loc", [NLOC, HID], f32)
            h_tab = nc.dram_tensor("h_tab", [TAB_ROWS, HID], f32,
                                   addr_space="Shared")

            zero64 = cpool.tile([64, HID], f32)
            nc.vector.memset(zero64[:], 0.0)

            # ---- scatter pass over tiles: layer = 1 or 2 ----
            def scatter_pass(layer, table):
                width = NODE_IN + EAW if layer == 1 else HID
                for w in range(NWIN):
                    acc = psum.tile([width, WIN], f32, space="PSUM", tag="acc")
                    for t in range(T_w):
                        kk = w * T_w + t
                        gath = pool.tile([128, G * 64], f32, tag=f"g{layer}")
                        for g in range(G):
                            nc.gpsimd.indirect_dma_start(
                                out=gath[:, g * 64:(g + 1) * 64],
                                out_offset=None,
                                in_=table[:],
                                in_offset=bass.IndirectOffsetOnAxis(
                                    ap=idx_sb[:, kk * G + g:kk * G + g + 1],
                                    axis=0),
                            )
                        comb = pool.tile([128, width], f32, tag=f"c{layer}")
                        nc.any.tensor_tensor(
                            out=comb[:, 0:64], in0=gath[:, 0:64],
                            in1=gath[:, 64:128], op=mybir.AluOpType.add)
                        if layer == 1:
                            eat = eapool.tile([128, G * EAW], f32, tag="ea")
                            nc.scalar.dma_start(
                                out=eat[:],
                                in_=ea_d[:, kk * G * EAW:(kk + 1) * G * EAW])
                            nc.any.tensor_tensor(
                                out=comb[:, 64:64 + EAW],
                                in0=eat[:, 0:EAW], in1=eat[:, EAW:2 * EAW],
                                op=mybir.AluOpType.add)
                        oh = pool.tile([128, 128], f32, tag="oh")
                        nc.vector.tensor_scalar(
                            out=oh[:], in0=iota_f[:],
                            scalar1=dst_sb[:, kk:kk + 1], scalar2=None,
                            op0=mybir.AluOpType.is_equal)
                        nc.tensor.matmul(acc[:], lhsT=comb[:], rhs=oh[:],
                                         start=(t == 0), stop=(t == T_w - 1))
                    dstT = aggt if layer == 1 else agg2t
                    nc.vector.tensor_copy(
                        out=dstT[0:width, w * WIN:(w + 1) * WIN], in_=acc[:])

            # ---- layer 1 ----
            scatter_pass(1, xtab)
            for w in range(NWIN):
                sl = slice(w * WIN, (w + 1) * WIN)
                f_ps = psumn.tile([HID, WIN], f32, space="PSUM", tag="fps")
                nc.tensor.matmul(f_ps[:], lhsT=w1x_sb[:], rhs=aggt[0:64, sl],
                                 start=True, stop=False)
                nc.tensor.matmul(f_ps[:], lhsT=w1s_sb[:], rhs=xt_sb[:, sl],
                                 start=False, stop=False)
                nc.tensor.matmul(f_ps[:],
                                 lhsT=w1sb_sb[NODE_IN:NODE_IN + EAW + 1, :],
                                 rhs=aggt[64:64 + EAW + 1, sl],
                                 start=False, stop=True)
                nc.scalar.activation(
                    out=ht_sb[:, sl], in_=f_ps[:],
                    func=mybir.ActivationFunctionType.Relu,
                    bias=bn_b_sb[:], scale=bn_a_sb[:])
                # node-major H tile for the gather table
                h_ps = psumt.tile([WIN, HID], f32, space="PSUM", tag="hps")
                nc.tensor.transpose(out=h_ps[:], in_=ht_sb[:, sl],
                                    identity=ident[0:HID, 0:HID])
                h_nm = pool.tile([WIN, HID], f32, tag="hnm")
                nc.vector.tensor_copy(out=h_nm[:], in_=h_ps[:])
                rows = min(WIN, NLOC - w * WIN)
                nc.scalar.dma_start(out=h_loc[w * WIN:w * WIN + rows, :],
                                    in_=h_nm[:rows, :])

            # ---- exchange H ----
            nc.gpsimd.collective_compute(
                "AllGather", mybir.AluOpType.bypass,
                replica_groups=[list(range(N_CORES))],
                ins=[h_loc[:, :]],
                outs=[h_tab[0:N, :]],
            )
            nc.sync.dma_start(out=h_tab[N:TAB_ROWS, :], in_=zero64[:])

            # ---- layer 2 ----
            scatter_pass(2, h_tab)
            for w in range(NWIN):
                sl = slice(w * WIN, (w + 1) * WIN)
                o_ps = psumn.tile([OUT, WIN], f32, space="PSUM", tag="fps")
                nc.tensor.matmul(o_ps[:], lhsT=w2x_sb[:], rhs=agg2t[:, sl],
                                 start=True, stop=False)
                nc.tensor.matmul(o_ps[:], lhsT=w2s_sb[:], rhs=ht_sb[:, sl],
                                 start=False, stop=False)
                nc.tensor.matmul(o_ps[:],
                                 lhsT=w2sb_sb[NODE_IN:NODE_IN + EAW + 1, :],
                                 rhs=aggt[64:64 + EAW + 1, sl],
                                 start=False, stop=True)
                ot_sb = pool.tile([OUT, WIN], f32, tag="ot")
                nc.vector.tensor_copy(out=ot_sb[:], in_=o_ps[:])
                o_ps2 = psumt.tile([WIN, OUT], f32, space="PSUM", tag="hps")
                nc.tensor.transpose(out=o_ps2[:], in_=ot_sb[:],
                                    identity=ident[0:OUT, 0:OUT])
                o_nm = pool.tile([WIN, OUT], f32, tag="onm")
                nc.vector.tensor_copy(out=o_nm[:], in_=o_ps2[:])
                rows = min(WIN, NLOC - w * WIN)
                nc.scalar.dma_start(out=out_d[w * WIN:w * WIN + rows, :],
                                    in_=o_nm[:rows, :])

    nc.compile()
    return nc


# ----------------------------------------------------------------------------
# entry point
# ----------------------------------------------------------------------------

def kernel(x, edge_index, edge_attr,
           W1_msg, b1_msg, W1_self, b1_self,
           bn_gamma, bn_beta, bn_mean, bn_var,
           W2_msg, b2_msg, W2_self, b2_self):
    from concourse.bass_utils import run_bass_kernel_spmd

    x = np.asarray(x, dtype=np.float32)
    idx_cols, dst_cols, ea_cols, T_w = _prep(np.asarray(edge_index),
                                             np.asarray(edge_attr))

    xtab = np.zeros((TAB_ROWS, NODE_IN), np.float32)
    xtab[:N] = x
    W1_msg = np.asarray(W1_msg, np.float32)
    W2_msg = np.asarray(W2_msg, np.float32)
    w1sb = np.concatenate([W1_msg[NODE_IN:], np.asarray(b1_msg, np.float32)[None, :],
                           np.asarray(b1_self, np.float32)[None, :]], axis=0)
    w2sb = np.concatenate([W2_msg[HID:], np.asarray(b2_msg, np.float32)[None, :],
                           np.asarray(b2_self, np.float32)[None, :]], axis=0)
    bn_a = (np.asarray(bn_gamma, np.float32)
            / np.sqrt(np.asarray(bn_var, np.float32) + BN_EPS))
    bn_b = np.asarray(bn_beta, np.float32) - np.asarray(bn_mean, np.float32) * bn_a

    in_maps = []
    for c in range(N_CORES):
        xt = np.zeros((NODE_IN, NLOC_PAD), np.float32)
        xt[:, :NLOC] = x[c * NLOC:(c + 1) * NLOC].T
        in_maps.append(dict(
            xtab=xtab, xt_loc=xt,
            idxs=idx_cols[c], dstm=dst_cols[c], ea=ea_cols[c],
            w1x=W1_msg[:NODE_IN], w1sb=w1sb, w1s=np.asarray(W1_self, np.float32),
            w2x=W2_msg[:HID], w2sb=w2sb, w2s=np.asarray(W2_self, np.float32),
            bn_a=bn_a[:, None], bn_b=bn_b[:, None],
        ))

    nc = _build(T_w)
    trace = os.environ.get("GNN_TRACE", "0") == "1"
    r = run_bass_kernel_spmd(nc, in_maps, list(range(N_CORES)), trace=trace)
    if trace:
        kernel.last_exec_time_ns = r.exec_time_ns
    out = np.concatenate([r.results[c]["out"] for c in range(N_CORES)], axis=0)
    return out
